# revision 47
# baseline (speedup 1.0000x reference)
"""Causal self-attention (B=2, T=2048, C=1024, H=16) on 8 trn2 NeuronCores.

Sharding: Megatron-style tensor parallel crossed with data parallel.
Core cid = 4*b + g handles batch b (of 2) and head group g (4 heads of 16).
Each core computes its 4 heads' attention plus the partial output
projection (w_proj rows for those heads); the host sums the 4 partials
per batch and adds b_proj. No device collectives needed.

Matmul operands are bf16 (inputs rounded on host), accumulation fp32 in
PSUM; softmax runs in fp32 (exp reads PSUM directly, denominators and
their reciprocals stay fp32/f32r). Everything stays in "transposed
space" so no on-device transposes are needed: the host passes x[b].T:
  - q^T/k^T come from  lhsT=w_qk[c,j],  rhs=xT[c,t]
  - V      comes from  lhsT=xT[c,t],    rhs=w_v[c,j]
  - S^T    comes from  lhsT=k^T[d,tk],  rhs=q^T[d,tq]   (d zero-padded to 128)
  - y^T    comes from  lhsT=V[tk,d|1],  rhs=P^T[tk,tq]  (ones col -> l)
  - out    comes from  lhsT=y^T[d,t],   rhs=w_proj[d,c]
Softmax skips max-subtraction (logits ~N(0,1), |s|<~7, exp safe in fp32);
causal masking multiplies the 4 diagonal blocks by precomputed staircase
masks after the exp. The softmax denominator l is harvested from a ones
column in V, reciprocals are batched per q-chunk ([4,512] fp32), and
1/l is partition-broadcast with a K=1 matmul so the normalize is a
single DVE multiply against PSUM.
"""

import numpy as np

B, T, C, H = 2, 2048, 1024, 16
HD = C // H  # 64
P = 128
NKT = C // P  # 8 k-tiles over the embedding dim
TCH = 512  # t-chunk (q) width
NCH = T // TCH  # 4 q-chunks
NTB = T // P  # 16 t-blocks (k) per sequence
HPC = 4  # heads per core
DC = HPC * HD  # 256 head dims per core

_CACHE = {}


def _build():
    import concourse.mybir as mybir
    from concourse import bacc
    from concourse.tile import TileContext

    F32 = mybir.dt.float32
    F32R = mybir.dt.float32r
    BF16 = mybir.dt.bfloat16
    AF = mybir.ActivationFunctionType

    nc = bacc.Bacc("TRN2", target_bir_lowering=False, debug=False)

    xT = nc.dram_tensor("xT", (C, T), BF16, kind="ExternalInput")
    wqk = nc.dram_tensor("wqk", (C, 2 * DC), BF16, kind="ExternalInput")
    wv = nc.dram_tensor("wv", (C, DC), BF16, kind="ExternalInput")
    wproj = nc.dram_tensor("wproj", (DC, C), BF16, kind="ExternalInput")
    bqk = nc.dram_tensor("bqk", (P, 4), F32, kind="ExternalInput")
    # bv extended with a ones column at [..., HD] (feeds V's l-sum column)
    bv = nc.dram_tensor("bv", (1, HPC, HD + 1), F32, kind="ExternalInput")
    # block-diagonal ones (rows {0,1} and duplicated at {32,33} so lhsT's
    # base partition can match the rhs slice): row -> out partition half
    ones2 = nc.dram_tensor("ones2", (34, P), F32R, kind="ExternalInput")
    masks = nc.dram_tensor("masks", (P, 4 * TCH), BF16, kind="ExternalInput")
    out = nc.dram_tensor("out", (T, C), F32, kind="ExternalOutput")

    with TileContext(nc) as tc:
        with (
            tc.tile_pool(name="persist", bufs=1) as pp,
            tc.tile_pool(name="consts", bufs=1) as cp,
        ):
            # ---- persistent SBUF ----
            wqk_sb = pp.tile([P, NKT, 2 * DC], BF16)  # 8KB/part
            wv_sb = pp.tile([P, NKT, DC], BF16)  # 4KB
            wproj_sb = pp.tile([P, DC // P, C], BF16)  # 4KB
            bqk_sb = cp.tile([P, 4], F32)
            bv_sb = cp.tile([P, HPC, HD + 1], F32)
            masks_sb = cp.tile([P, 4 * TCH], BF16)
            ones_sb = cp.tile([34, P], F32R)
            # per-head q^T / k^T, rows 64:128 zero so S runs at K=128
            qts = [
                pp.tile([P, T], BF16, tag=f"qt{h}", name=f"qt{h}")
                for h in range(HPC)
            ]
            kts = [
                pp.tile([P, T], BF16, tag=f"kt{h}", name=f"kt{h}")
                for h in range(HPC)
            ]
            v_sb = pp.tile([P, NTB, HPC, HD + 1], BF16)  # 8.1KB (+ones col)
            yT_sb = pp.tile([P, DC // P, T], BF16)  # 8KB
            # l rows at partitions {0,1,32,33}: matmul bases must be 0/32/64,
            # engine writes must be 32-aligned (odd rows arrive via DMA).
            # Unused rows are memset to 1.0 so the whole-tile recip is finite.
            l4_sb = cp.tile([64, TCH], F32)
            rec4_sb = cp.tile([64, TCH], F32R)  # 1/l

            # NOTE: DMA engines drain their queues FIFO, so emission order
            # is completion-priority order. Critical startup set: x(0), wqk.
            wqk_r = wqk[:].rearrange("(kt p) j -> p kt j", p=P)
            wv_r = wv[:].rearrange("(kt p) j -> p kt j", p=P)
            wproj_r = wproj[:].rearrange("(kt p) n -> p kt n", p=P)
            for h in range(HPC):
                nc.vector.memset(qts[h][HD:P, :], 0.0)
                nc.vector.memset(kts[h][HD:P, :], 0.0)
            nc.vector.memset(l4_sb[:], 1.0)

            xT_r = xT[:].rearrange("(kt p) t -> p kt t", p=P)

            # ---- fused emission: attention(a) is ACT(exp)-bound, so the
            # in-order PE stream is stuffed with "filler" matmul steps from
            # QKV(a+1) and proj(a-1) between attention blocks. Attention for
            # chunk a only needs QKV from chunks <= a.
            with (
                tc.tile_pool(name="xin", bufs=2) as xpool,
                tc.tile_pool(name="ps_s", bufs=4, space="PSUM") as ps_s,
                tc.tile_pool(name="ps_y", bufs=2, space="PSUM") as ps_y,
                tc.tile_pool(name="ps_o", bufs=2, space="PSUM") as ps_o,
                tc.tile_pool(name="pt", bufs=10) as ptp,
                tc.tile_pool(name="outs", bufs=3) as outp,
                tc.tile_pool(name="lt", bufs=2) as ltp,
            ):

                def make_qkv_steps(a, x_tile=None):
                    """Emit x DMA now; return half-group matmul steps."""
                    ch = slice(a * TCH, (a + 1) * TCH)
                    if x_tile is None:
                        x_tile = xpool.tile(
                            [P, NKT, TCH], BF16, tag="x_tile", name=f"x{a}"
                        )
                        nc.sync.dma_start(x_tile[:, 0:4, :], xT_r[:, 0:4, ch])
                        nc.sync.dma_start(x_tile[:, 4:NKT, :], xT_r[:, 4:NKT, ch])
                    state = {}
                    steps = []

                    def qk1(jt):
                        pq = ps_s.tile([P, TCH], F32, tag="ps", name=f"pq{a}_{jt}")
                        for kt in range(NKT):
                            nc.tensor.matmul(
                                pq[:],
                                wqk_sb[:, kt, jt * P : (jt + 1) * P],
                                x_tile[:, kt, :],
                                start=(kt == 0),
                                stop=(kt == NKT - 1),
                            )
                        # out = in*scale + bias; q tiles carry the 1/sqrt(hd)
                        # scale (host pre-scaled the q bias)
                        dsts = (
                            (qts[2 * jt], qts[2 * jt + 1])
                            if jt < 2
                            else (kts[2 * (jt - 2)], kts[2 * (jt - 2) + 1])
                        )
                        sc = 0.125 if jt < 2 else 1.0
                        for lane in range(2):
                            nc.vector.tensor_scalar(
                                dsts[lane][0:HD, ch],
                                pq[lane * HD : (lane + 1) * HD, :],
                                sc,
                                bqk_sb[lane * HD : (lane + 1) * HD, jt : jt + 1],
                                mybir.AluOpType.mult,
                                mybir.AluOpType.add,
                            )

                    def v1(tb):
                        pv = ps_y.tile(
                            [P, HPC, HD], F32, tag="py", name=f"pv{a}_{tb}"
                        )
                        tg = a * (TCH // P) + tb
                        for kt in range(NKT):
                            nc.tensor.matmul(
                                pv[:],
                                x_tile[:, kt, tb * P : (tb + 1) * P],
                                wv_sb[:, kt, :],
                                start=(kt == 0),
                                stop=(kt == NKT - 1),
                            )
                        nc.vector.tensor_add(
                            v_sb[:, tg, :, 0:HD], pv[:], bv_sb[:, :, 0:HD]
                        )
                        # ones column for the softmax-denominator row of AV
                        nc.vector.tensor_copy(
                            v_sb[:, tg, :, HD : HD + 1], bv_sb[:, :, HD : HD + 1]
                        )

                    for jt in range(4):
                        steps.append(lambda jt=jt: qk1(jt))
                    for tb in range(4):
                        steps.append(lambda tb=tb: v1(tb))
                    return steps

                def proj_steps(a):
                    """Projection of chunk a: 8 steps of [2 MMs + copy + DMA]."""
                    steps = []

                    def pstep(tb, ncx):
                        tg = a * (TCH // P) + tb
                        po = ps_o.tile(
                            [P, TCH], F32, tag="po", name=f"po{a}_{tb}_{ncx}"
                        )
                        for kt in range(DC // P):
                            nc.tensor.matmul(
                                po[:],
                                yT_sb[:, kt, tg * P : (tg + 1) * P],
                                wproj_sb[:, kt, ncx * TCH : (ncx + 1) * TCH],
                                start=(kt == 0),
                                stop=(kt == DC // P - 1),
                            )
                        o_tile = outp.tile([P, TCH], F32, tag="osb")
                        if ncx == 0:
                            nc.scalar.copy(o_tile[:], po[:])
                        else:
                            nc.vector.tensor_copy(o_tile[:], po[:])
                        nc.sync.dma_start(
                            out[tg * P : (tg + 1) * P, ncx * TCH : (ncx + 1) * TCH],
                            o_tile[:],
                        )

                    for tb in range(TCH // P):
                        for ncx in range(2):
                            steps.append(lambda tb=tb, ncx=ncx: pstep(tb, ncx))
                    return steps

                # prologue, DMA-priority ordered: x(0) first, then wqk and
                # the small bias/ones tensors, then wv; masks and wproj are
                # deferred (needed only at attention(0) / proj(0))
                x0 = xpool.tile([P, NKT, TCH], BF16, tag="x_tile", name="x0")
                nc.sync.dma_start(x0[:, 0:4, :], xT_r[:, 0:4, 0:TCH])
                for kt in range(4):
                    nc.sync.dma_start(wqk_sb[:, kt, :], wqk_r[:, kt, :])
                nc.sync.dma_start(x0[:, 4:NKT, :], xT_r[:, 4:NKT, 0:TCH])
                for kt in range(4, NKT):
                    nc.sync.dma_start(wqk_sb[:, kt, :], wqk_r[:, kt, :])
                nc.sync.dma_start(bqk_sb[:], bqk[:])
                nc.sync.dma_start(bv_sb[:], bv[:].to_broadcast((P, HPC, HD + 1)))
                nc.sync.dma_start(ones_sb[:], ones2[:])
                for kt in range(NKT):
                    nc.sync.dma_start(wv_sb[:, kt, :], wv_r[:, kt, :])
                # prologue: QKV of chunk 0 runs un-overlapped
                steps0 = make_qkv_steps(0, x_tile=x0)
                for st in steps0[:6]:
                    st()
                nc.sync.dma_start(masks_sb[:], masks[:])
                for kt in range(DC // P):
                    nc.sync.dma_start(wproj_sb[:, kt, :], wproj_r[:, kt, :])
                for st in steps0[6:]:
                    st()

                for a in range(NCH):
                    ch = slice(a * TCH, (a + 1) * TCH)
                    fillers = []
                    if a + 1 < NCH:
                        fillers += make_qkv_steps(a + 1)
                    # proj fillers pushed late, where attention is ACT-bound
                    # and has few QKV fillers left
                    if a == 2:
                        fillers += proj_steps(0)
                    elif a == 3:
                        fillers += proj_steps(1) + proj_steps(2)
                    nblk = 4 * a + 4  # causal: k-blocks 0..4a+3
                    for h in range(HPC):
                        py = ps_y.tile(
                            [HD + 1, TCH], F32, tag="py", name=f"py{a}_{h}"
                        )
                        # software-pipeline: AV(j) is enqueued after S(j+3) so
                        # the in-order PE never stalls waiting for exp(j)
                        DEPTH = 4
                        pts = {}

                        def emit_s(j, h=h, a=a):
                            r = j - 4 * a
                            # diagonal blocks: cols t_q < 128*r are fully
                            # masked — skip them in S, exp, mask and AV
                            c0 = 128 * r if r > 0 else 0
                            ps = ps_s.tile([P, TCH], F32, tag="ps")
                            nc.tensor.matmul(
                                ps[:, c0:],
                                kts[h][:, j * P : (j + 1) * P],
                                qts[h][:, a * TCH + c0 : (a + 1) * TCH],
                                start=True,
                                stop=True,
                            )
                            pt = ptp.tile([P, TCH], BF16)
                            nc.scalar.activation(pt[:, c0:], ps[:, c0:], AF.Exp)
                            if r >= 0:
                                nc.gpsimd.tensor_mul(
                                    pt[:, c0:],
                                    pt[:, c0:],
                                    masks_sb[:, r * TCH + c0 : (r + 1) * TCH],
                                )
                            pts[j] = (pt, c0)

                        def emit_av(j, h=h, py=py, nblk=nblk):
                            pt, c0 = pts.pop(j)
                            nc.tensor.matmul(
                                py[:, c0:],
                                v_sb[:, j, h, :],
                                pt[:, c0:],
                                start=(j == 0),
                                stop=(j == nblk - 1),
                            )

                        for j in range(nblk):
                            emit_s(j)
                            if fillers:
                                fillers.pop(0)()
                            if j >= DEPTH:
                                emit_av(j - DEPTH)
                        for j in range(max(0, nblk - DEPTH), nblk):
                            emit_av(j)
                        # stash unnormalized y^T and the denominator row,
                        # freeing the AV psum bank
                        nc.vector.tensor_copy(
                            yT_sb[64 * (h % 2) : 64 * (h % 2) + 64, h // 2, ch],
                            py[0:HD, :],
                        )
                        lrow = (h % 2) + 32 * (h // 2)
                        if h % 2 == 0:
                            nc.vector.tensor_copy(
                                l4_sb[lrow : lrow + 1, :], py[HD : HD + 1, :]
                            )
                        else:
                            # engines can't write partition 1/33; stage via
                            # partition 0 then hop with a tiny SBUF->SBUF DMA
                            lt = ltp.tile([1, TCH], F32)
                            nc.vector.tensor_copy(lt[:], py[HD : HD + 1, :])
                            nc.sync.dma_start(l4_sb[lrow : lrow + 1, :], lt[:])
                            # pair done: reciprocal + broadcast (K=2 matmul
                            # with block-diag ones: row0 -> out partitions
                            # 0:64, row1 -> 64:128) + normalize in place
                            hp = h // 2
                            with nc.allow_low_precision(
                                reason="f32r recip feeds bcast matmul; l>=1"
                            ):
                                nc.vector.reciprocal(
                                    rec4_sb[32 * hp : 32 * hp + 2, :],
                                    l4_sb[32 * hp : 32 * hp + 2, :],
                                )
                            rb = ps_o.tile([P, TCH], F32, tag="po")
                            nc.tensor.matmul(
                                rb[:],
                                ones_sb[32 * hp : 32 * hp + 2, :],
                                rec4_sb[32 * hp : 32 * hp + 2, :],
                                start=True,
                                stop=True,
                            )
                            ysl = yT_sb[:, hp, ch]
                            nc.vector.tensor_mul(ysl, ysl, rb[:])
                    # leftover fillers for this chunk
                    for st in fillers:
                        st()
                # epilogue: projection of the last chunk
                for st in proj_steps(NCH - 1):
                    st()

    nc.compile()
    return nc


def _ones2():
    o = np.zeros((34, P), np.float32)
    blk = np.kron(np.eye(2, dtype=np.float32), np.ones((1, 64), np.float32))
    o[0:2] = blk
    o[32:34] = blk
    return o


def _in_maps(x, w_attn, b_attn, w_proj):
    """Build the 8 per-core input maps (cid = 4*b + g)."""
    import ml_dtypes

    bf16 = ml_dtypes.bfloat16
    f = np.arange(4 * TCH) % TCH
    r = np.arange(4 * TCH) // TCH
    p = np.arange(P)
    masks = (p[:, None] <= (f - P * r)[None, :]).astype(bf16)

    wq, wk, wvv = w_attn[:, 0:C], w_attn[:, C : 2 * C], w_attn[:, 2 * C : 3 * C]
    bq, bk, bvv = b_attn[0:C], b_attn[C : 2 * C], b_attn[2 * C : 3 * C]

    maps = []
    for b in range(B):
        xTb = np.ascontiguousarray(x[b].T.astype(bf16))
        for g in range(4):
            s = slice(g * DC, (g + 1) * DC)
            wqk_c = np.ascontiguousarray(
                np.concatenate([wq[:, s], wk[:, s]], axis=1).astype(bf16)
            )
            bqk_c = np.stack(
                [
                    0.125 * bq[s][0:P],
                    0.125 * bq[s][P:DC],
                    bk[s][0:P],
                    bk[s][P:DC],
                ],
                axis=1,
            ).astype(np.float32)
            maps.append(
                {
                    "xT": xTb,
                    "wqk": wqk_c,
                    "wv": np.ascontiguousarray(wvv[:, s].astype(bf16)),
                    "wproj": np.ascontiguousarray(w_proj[s, :].astype(bf16)),
                    "bqk": np.ascontiguousarray(bqk_c),
                    "bv": np.ascontiguousarray(
                        np.concatenate(
                            [
                                bvv[s].reshape(HPC, HD),
                                np.ones((HPC, 1), np.float32),
                            ],
                            axis=1,
                        ).reshape(1, HPC, HD + 1).astype(np.float32)
                    ),
                    "ones2": _ones2(),
                    "masks": masks,
                }
            )
    return maps


def run(x, w_attn, b_attn, w_proj, b_proj, trace=False):
    from concourse.bass_utils import run_bass_kernel_spmd

    if "nc" not in _CACHE:
        _CACHE["nc"] = _build()
    nc = _CACHE["nc"]
    maps = _in_maps(
        np.asarray(x), np.asarray(w_attn), np.asarray(b_attn), np.asarray(w_proj)
    )
    r = run_bass_kernel_spmd(nc, maps, core_ids=list(range(8)), trace=trace)
    partials = [r.results[i]["out"] for i in range(8)]
    bp = np.asarray(b_proj, dtype=np.float32)
    y = np.stack(
        [sum(partials[4 * b : 4 * b + 4]) + bp for b in range(B)], axis=0
    ).astype(np.float32)
    return y, r


def kernel(x, w_attn, b_attn, w_proj, b_proj):
    y, _ = run(x, w_attn, b_attn, w_proj, b_proj, trace=False)
    return y


# revision 49
# speedup vs baseline: 1.0142x; 1.0142x over previous
"""Causal self-attention (B=2, T=2048, C=1024, H=16) on 8 trn2 NeuronCores.

Sharding: Megatron-style tensor parallel crossed with data parallel.
Core cid = 4*b + g handles batch b (of 2) and head group g (4 heads of 16).
Each core computes its 4 heads' attention plus the partial output
projection (w_proj rows for those heads); the host sums the 4 partials
per batch and adds b_proj. No device collectives needed.

Matmul operands are bf16 (inputs rounded on host), accumulation fp32 in
PSUM; softmax runs in fp32 (exp reads PSUM directly, denominators and
their reciprocals stay fp32/f32r). Everything stays in "transposed
space" so no on-device transposes are needed: the host passes x[b].T:
  - q^T/k^T come from  lhsT=w_qk[c,j],  rhs=xT[c,t]
  - V      comes from  lhsT=xT[c,t],    rhs=w_v[c,j]
  - S^T    comes from  lhsT=k^T[d,tk],  rhs=q^T[d,tq]   (d zero-padded to 128)
  - y^T    comes from  lhsT=V[tk,d|1],  rhs=P^T[tk,tq]  (ones col -> l)
  - out    comes from  lhsT=y^T[d,t],   rhs=w_proj[d,c]
Softmax skips max-subtraction (logits ~N(0,1), |s|<~7, exp safe in fp32);
causal masking multiplies the 4 diagonal blocks by precomputed staircase
masks after the exp. The softmax denominator l is harvested from a ones
column in V, reciprocals are batched per q-chunk ([4,512] fp32), and
1/l is partition-broadcast with a K=1 matmul so the normalize is a
single DVE multiply against PSUM.
"""

import numpy as np

B, T, C, H = 2, 2048, 1024, 16
HD = C // H  # 64
P = 128
NKT = C // P  # 8 k-tiles over the embedding dim
TCH = 512  # t-chunk (q) width
NCH = T // TCH  # 4 q-chunks
NTB = T // P  # 16 t-blocks (k) per sequence
HPC = 4  # heads per core
DC = HPC * HD  # 256 head dims per core

_CACHE = {}


def _build():
    import concourse.mybir as mybir
    from concourse import bacc
    from concourse.tile import TileContext

    F32 = mybir.dt.float32
    F32R = mybir.dt.float32r
    BF16 = mybir.dt.bfloat16
    AF = mybir.ActivationFunctionType

    nc = bacc.Bacc("TRN2", target_bir_lowering=False, debug=False)

    xT = nc.dram_tensor("xT", (C, T), BF16, kind="ExternalInput")
    wqk = nc.dram_tensor("wqk", (C, 2 * DC), BF16, kind="ExternalInput")
    wv = nc.dram_tensor("wv", (C, DC), BF16, kind="ExternalInput")
    wproj = nc.dram_tensor("wproj", (DC, C), BF16, kind="ExternalInput")
    bqk = nc.dram_tensor("bqk", (P, 4), F32, kind="ExternalInput")
    # bv extended with a ones column at [..., HD] (feeds V's l-sum column)
    bv = nc.dram_tensor("bv", (1, HPC, HD + 1), F32, kind="ExternalInput")
    # block-diagonal ones (rows {0,1} and duplicated at {32,33} so lhsT's
    # base partition can match the rhs slice): row -> out partition half
    ones2 = nc.dram_tensor("ones2", (34, P), F32R, kind="ExternalInput")
    masks = nc.dram_tensor("masks", (P, 4 * TCH), BF16, kind="ExternalInput")
    out = nc.dram_tensor("out", (T, C), F32, kind="ExternalOutput")

    with TileContext(nc) as tc:
        with (
            tc.tile_pool(name="persist", bufs=1) as pp,
            tc.tile_pool(name="consts", bufs=1) as cp,
        ):
            # ---- persistent SBUF ----
            wqk_sb = pp.tile([P, NKT, 2 * DC], BF16)  # 8KB/part
            wv_sb = pp.tile([P, NKT, DC], BF16)  # 4KB
            wproj_sb = pp.tile([P, DC // P, C], BF16)  # 4KB
            bqk_sb = cp.tile([P, 4], F32)
            bv_sb = cp.tile([P, HPC, HD + 1], F32)
            masks_sb = cp.tile([P, 4 * TCH], BF16)
            ones_sb = cp.tile([34, P], F32R)
            # per-head q^T / k^T, rows 64:128 zero so S runs at K=128
            qts = [
                pp.tile([P, T], BF16, tag=f"qt{h}", name=f"qt{h}")
                for h in range(HPC)
            ]
            kts = [
                pp.tile([P, T], BF16, tag=f"kt{h}", name=f"kt{h}")
                for h in range(HPC)
            ]
            v_sb = pp.tile([P, NTB, HPC, HD + 1], BF16)  # 8.1KB (+ones col)
            yT_sb = pp.tile([P, DC // P, T], BF16)  # 8KB
            # l rows at partitions {0,1,32,33}: matmul bases must be 0/32/64,
            # engine writes must be 32-aligned (odd rows arrive via DMA).
            # Unused rows are memset to 1.0 so the whole-tile recip is finite.
            l4_sb = cp.tile([64, TCH], F32)
            rec4_sb = cp.tile([64, TCH], F32R)  # 1/l

            # NOTE: DMA engines drain their queues FIFO, so emission order
            # is completion-priority order. Critical startup set: x(0), wqk.
            wqk_r = wqk[:].rearrange("(kt p) j -> p kt j", p=P)
            wv_r = wv[:].rearrange("(kt p) j -> p kt j", p=P)
            wproj_r = wproj[:].rearrange("(kt p) n -> p kt n", p=P)
            for h in range(HPC):
                nc.vector.memset(qts[h][HD:P, :], 0.0)
                nc.vector.memset(kts[h][HD:P, :], 0.0)
            nc.vector.memset(l4_sb[:], 1.0)

            xT_r = xT[:].rearrange("(kt p) t -> p kt t", p=P)

            # ---- fused emission: attention(a) is ACT(exp)-bound, so the
            # in-order PE stream is stuffed with "filler" matmul steps from
            # QKV(a+1) and proj(a-1) between attention blocks. Attention for
            # chunk a only needs QKV from chunks <= a.
            with (
                tc.tile_pool(name="xin", bufs=2) as xpool,
                tc.tile_pool(name="ps_s", bufs=4, space="PSUM") as ps_s,
                tc.tile_pool(name="ps_y", bufs=2, space="PSUM") as ps_y,
                tc.tile_pool(name="ps_o", bufs=2, space="PSUM") as ps_o,
                tc.tile_pool(name="pt", bufs=10) as ptp,
                tc.tile_pool(name="outs", bufs=3) as outp,
                tc.tile_pool(name="lt", bufs=2) as ltp,
            ):

                def make_qkv_steps(a, x_tile=None):
                    """Emit x DMA now; return half-group matmul steps."""
                    ch = slice(a * TCH, (a + 1) * TCH)
                    if x_tile is None:
                        x_tile = xpool.tile(
                            [P, NKT, TCH], BF16, tag="x_tile", name=f"x{a}"
                        )
                        nc.sync.dma_start(x_tile[:, 0:4, :], xT_r[:, 0:4, ch])
                        nc.sync.dma_start(x_tile[:, 4:NKT, :], xT_r[:, 4:NKT, ch])
                    state = {}
                    steps = []

                    def qk1(jt):
                        pq = ps_s.tile([P, TCH], F32, tag="ps", name=f"pq{a}_{jt}")
                        for kt in range(NKT):
                            nc.tensor.matmul(
                                pq[:],
                                wqk_sb[:, kt, jt * P : (jt + 1) * P],
                                x_tile[:, kt, :],
                                start=(kt == 0),
                                stop=(kt == NKT - 1),
                            )
                        # out = in*scale + bias; q tiles carry the 1/sqrt(hd)
                        # scale (host pre-scaled the q bias)
                        dsts = (
                            (qts[2 * jt], qts[2 * jt + 1])
                            if jt < 2
                            else (kts[2 * (jt - 2)], kts[2 * (jt - 2) + 1])
                        )
                        sc = 0.125 if jt < 2 else 1.0
                        for lane in range(2):
                            nc.vector.tensor_scalar(
                                dsts[lane][0:HD, ch],
                                pq[lane * HD : (lane + 1) * HD, :],
                                sc,
                                bqk_sb[lane * HD : (lane + 1) * HD, jt : jt + 1],
                                mybir.AluOpType.mult,
                                mybir.AluOpType.add,
                            )

                    def v1(tb):
                        pv = ps_y.tile(
                            [P, HPC, HD], F32, tag="py", name=f"pv{a}_{tb}"
                        )
                        tg = a * (TCH // P) + tb
                        for kt in range(NKT):
                            nc.tensor.matmul(
                                pv[:],
                                x_tile[:, kt, tb * P : (tb + 1) * P],
                                wv_sb[:, kt, :],
                                start=(kt == 0),
                                stop=(kt == NKT - 1),
                            )
                        nc.vector.tensor_add(
                            v_sb[:, tg, :, 0:HD], pv[:], bv_sb[:, :, 0:HD]
                        )
                        # ones column for the softmax-denominator row of AV
                        nc.vector.tensor_copy(
                            v_sb[:, tg, :, HD : HD + 1], bv_sb[:, :, HD : HD + 1]
                        )

                    for jt in range(4):
                        steps.append(lambda jt=jt: qk1(jt))
                    for tb in range(4):
                        steps.append(lambda tb=tb: v1(tb))
                    return steps

                def proj_steps(a):
                    """Projection of chunk a: 8 steps of [2 MMs + copy + DMA]."""
                    steps = []

                    def pstep(tb, ncx):
                        tg = a * (TCH // P) + tb
                        po = ps_o.tile(
                            [P, TCH], F32, tag="po", name=f"po{a}_{tb}_{ncx}"
                        )
                        for kt in range(DC // P):
                            nc.tensor.matmul(
                                po[:],
                                yT_sb[:, kt, tg * P : (tg + 1) * P],
                                wproj_sb[:, kt, ncx * TCH : (ncx + 1) * TCH],
                                start=(kt == 0),
                                stop=(kt == DC // P - 1),
                            )
                        o_tile = outp.tile([P, TCH], F32, tag="osb")
                        if ncx == 0:
                            nc.scalar.copy(o_tile[:], po[:])
                        else:
                            nc.vector.tensor_copy(o_tile[:], po[:])
                        nc.sync.dma_start(
                            out[tg * P : (tg + 1) * P, ncx * TCH : (ncx + 1) * TCH],
                            o_tile[:],
                        )

                    for tb in range(TCH // P):
                        for ncx in range(2):
                            steps.append(lambda tb=tb, ncx=ncx: pstep(tb, ncx))
                    return steps

                # prologue, DMA-priority ordered: x(0) first, then wqk and
                # the small bias/ones tensors, then wv; masks and wproj are
                # deferred (needed only at attention(0) / proj(0))
                x0 = xpool.tile([P, NKT, TCH], BF16, tag="x_tile", name="x0")
                nc.sync.dma_start(x0[:, 0:4, :], xT_r[:, 0:4, 0:TCH])
                for kt in range(4):
                    nc.sync.dma_start(wqk_sb[:, kt, :], wqk_r[:, kt, :])
                nc.sync.dma_start(x0[:, 4:NKT, :], xT_r[:, 4:NKT, 0:TCH])
                for kt in range(4, NKT):
                    nc.sync.dma_start(wqk_sb[:, kt, :], wqk_r[:, kt, :])
                nc.sync.dma_start(bqk_sb[:], bqk[:])
                nc.sync.dma_start(bv_sb[:], bv[:].to_broadcast((P, HPC, HD + 1)))
                nc.sync.dma_start(ones_sb[:], ones2[:])
                for kt in range(NKT):
                    nc.sync.dma_start(wv_sb[:, kt, :], wv_r[:, kt, :])
                # prologue: only what attention(0) heads 0/1 need up front
                # (q/k j-tiles 0 and 2 + V); j-tiles 1/3 become att(0) fillers
                steps0 = make_qkv_steps(0, x_tile=x0)
                steps0[0]()
                steps0[2]()
                nc.sync.dma_start(masks_sb[:], masks[:])
                for kt in range(DC // P):
                    nc.sync.dma_start(wproj_sb[:, kt, :], wproj_r[:, kt, :])
                for st in steps0[4:8]:
                    st()
                leftover0 = [steps0[1], steps0[3]]

                for a in range(NCH):
                    ch = slice(a * TCH, (a + 1) * TCH)
                    fillers = leftover0 if a == 0 else []
                    if a + 1 < NCH:
                        fillers += make_qkv_steps(a + 1)
                    # proj fillers pushed late, where attention is ACT-bound
                    # and has few QKV fillers left
                    if a == 2:
                        fillers += proj_steps(0)
                    elif a == 3:
                        fillers += proj_steps(1) + proj_steps(2)
                    nblk = 4 * a + 4  # causal: k-blocks 0..4a+3
                    for h in range(HPC):
                        py = ps_y.tile(
                            [HD + 1, TCH], F32, tag="py", name=f"py{a}_{h}"
                        )
                        # software-pipeline: AV(j) is enqueued after S(j+3) so
                        # the in-order PE never stalls waiting for exp(j)
                        DEPTH = 4
                        pts = {}

                        def emit_s(j, h=h, a=a):
                            r = j - 4 * a
                            # diagonal blocks: cols t_q < 128*r are fully
                            # masked — skip them in S, exp, mask and AV
                            c0 = 128 * r if r > 0 else 0
                            ps = ps_s.tile([P, TCH], F32, tag="ps")
                            nc.tensor.matmul(
                                ps[:, c0:],
                                kts[h][:, j * P : (j + 1) * P],
                                qts[h][:, a * TCH + c0 : (a + 1) * TCH],
                                start=True,
                                stop=True,
                            )
                            pt = ptp.tile([P, TCH], BF16)
                            nc.scalar.activation(pt[:, c0:], ps[:, c0:], AF.Exp)
                            if r >= 0:
                                nc.gpsimd.tensor_mul(
                                    pt[:, c0:],
                                    pt[:, c0:],
                                    masks_sb[:, r * TCH + c0 : (r + 1) * TCH],
                                )
                            pts[j] = (pt, c0)

                        def emit_av(j, h=h, py=py, nblk=nblk):
                            pt, c0 = pts.pop(j)
                            nc.tensor.matmul(
                                py[:, c0:],
                                v_sb[:, j, h, :],
                                pt[:, c0:],
                                start=(j == 0),
                                stop=(j == nblk - 1),
                            )

                        for j in range(nblk):
                            emit_s(j)
                            if fillers:
                                fillers.pop(0)()
                            if j >= DEPTH:
                                emit_av(j - DEPTH)
                        for j in range(max(0, nblk - DEPTH), nblk):
                            emit_av(j)
                        # stash unnormalized y^T and the denominator row,
                        # freeing the AV psum bank
                        nc.vector.tensor_copy(
                            yT_sb[64 * (h % 2) : 64 * (h % 2) + 64, h // 2, ch],
                            py[0:HD, :],
                        )
                        lrow = (h % 2) + 32 * (h // 2)
                        if h % 2 == 0:
                            nc.vector.tensor_copy(
                                l4_sb[lrow : lrow + 1, :], py[HD : HD + 1, :]
                            )
                        else:
                            # engines can't write partition 1/33; stage via
                            # partition 0 then hop with a tiny SBUF->SBUF DMA
                            lt = ltp.tile([1, TCH], F32)
                            nc.vector.tensor_copy(lt[:], py[HD : HD + 1, :])
                            nc.sync.dma_start(l4_sb[lrow : lrow + 1, :], lt[:])
                            # pair done: defer the reciprocal + broadcast
                            # (K=2 matmul with block-diag ones) + normalize
                            # into the filler stream so the in-order PE
                            # doesn't stall on the l-hop DMA + recip chain

                            def norm_step(hp=h // 2, ch=ch):
                                with nc.allow_low_precision(
                                    reason="f32r recip feeds bcast mm; l>=1"
                                ):
                                    nc.vector.reciprocal(
                                        rec4_sb[32 * hp : 32 * hp + 2, :],
                                        l4_sb[32 * hp : 32 * hp + 2, :],
                                    )
                                rb = ps_o.tile([P, TCH], F32, tag="po")
                                nc.tensor.matmul(
                                    rb[:],
                                    ones_sb[32 * hp : 32 * hp + 2, :],
                                    rec4_sb[32 * hp : 32 * hp + 2, :],
                                    start=True,
                                    stop=True,
                                )
                                ysl = yT_sb[:, hp, ch]
                                nc.vector.tensor_mul(ysl, ysl, rb[:])

                            fillers.insert(min(2, len(fillers)), norm_step)
                    # leftover fillers for this chunk
                    for st in fillers:
                        st()
                # epilogue: projection of the last chunk
                for st in proj_steps(NCH - 1):
                    st()

    nc.compile()
    return nc


def _ones2():
    o = np.zeros((34, P), np.float32)
    blk = np.kron(np.eye(2, dtype=np.float32), np.ones((1, 64), np.float32))
    o[0:2] = blk
    o[32:34] = blk
    return o


def _in_maps(x, w_attn, b_attn, w_proj):
    """Build the 8 per-core input maps (cid = 4*b + g)."""
    import ml_dtypes

    bf16 = ml_dtypes.bfloat16
    f = np.arange(4 * TCH) % TCH
    r = np.arange(4 * TCH) // TCH
    p = np.arange(P)
    masks = (p[:, None] <= (f - P * r)[None, :]).astype(bf16)

    wq, wk, wvv = w_attn[:, 0:C], w_attn[:, C : 2 * C], w_attn[:, 2 * C : 3 * C]
    bq, bk, bvv = b_attn[0:C], b_attn[C : 2 * C], b_attn[2 * C : 3 * C]

    maps = []
    for b in range(B):
        xTb = np.ascontiguousarray(x[b].T.astype(bf16))
        for g in range(4):
            s = slice(g * DC, (g + 1) * DC)
            wqk_c = np.ascontiguousarray(
                np.concatenate([wq[:, s], wk[:, s]], axis=1).astype(bf16)
            )
            bqk_c = np.stack(
                [
                    0.125 * bq[s][0:P],
                    0.125 * bq[s][P:DC],
                    bk[s][0:P],
                    bk[s][P:DC],
                ],
                axis=1,
            ).astype(np.float32)
            maps.append(
                {
                    "xT": xTb,
                    "wqk": wqk_c,
                    "wv": np.ascontiguousarray(wvv[:, s].astype(bf16)),
                    "wproj": np.ascontiguousarray(w_proj[s, :].astype(bf16)),
                    "bqk": np.ascontiguousarray(bqk_c),
                    "bv": np.ascontiguousarray(
                        np.concatenate(
                            [
                                bvv[s].reshape(HPC, HD),
                                np.ones((HPC, 1), np.float32),
                            ],
                            axis=1,
                        ).reshape(1, HPC, HD + 1).astype(np.float32)
                    ),
                    "ones2": _ones2(),
                    "masks": masks,
                }
            )
    return maps


def run(x, w_attn, b_attn, w_proj, b_proj, trace=False):
    from concourse.bass_utils import run_bass_kernel_spmd

    if "nc" not in _CACHE:
        _CACHE["nc"] = _build()
    nc = _CACHE["nc"]
    maps = _in_maps(
        np.asarray(x), np.asarray(w_attn), np.asarray(b_attn), np.asarray(w_proj)
    )
    r = run_bass_kernel_spmd(nc, maps, core_ids=list(range(8)), trace=trace)
    partials = [r.results[i]["out"] for i in range(8)]
    bp = np.asarray(b_proj, dtype=np.float32)
    y = np.stack(
        [sum(partials[4 * b : 4 * b + 4]) + bp for b in range(B)], axis=0
    ).astype(np.float32)
    return y, r


def kernel(x, w_attn, b_attn, w_proj, b_proj):
    y, _ = run(x, w_attn, b_attn, w_proj, b_proj, trace=False)
    return y
